# revision 1
# baseline (speedup 1.0000x reference)
"""NNUE feature-transformer + MLP head kernel for 8 Trainium2 NeuronCores.

Strategy (hardcoded for B=4096, F=40960, FT_OUT=257, 8 cores):
  - Data-parallel over batch: each core handles 512 batch rows end-to-end.
  - Host prep: transpose masks to [F, 512] per core and cast to fp16
    (0/1 masks are exact in fp16; ft_w fp16 adds ~2e-4 rel err), transpose
    ft_w to [F, 257] fp16.
  - Device: feature-transformer GEMM with mask tiles as the stationary
    operand ([128 feat x 128 batch]) and ft_w.T tiles [128, 257] streaming,
    accumulating into 8 PSUM banks (4 w-tiles + 4 b-tiles of [128, 257] f32)
    over 320 K-slices. Tiny epilogue (PE transposes, stm select, crelu,
    3-layer MLP, PSQT) on device. Output [1, 512] f32 per core.
"""

import os
import numpy as np
from contextlib import ExitStack

B = 4096
F = 40960
O = 257  # 256 accumulator + 1 PSQT
NCORES = 8
BC = B // NCORES  # 512 batch rows per core
# Feature chunk schedule: small head chunks to shorten the pipeline ramp,
# then 4MB (4096-feature) mask DMAs for peak HBM efficiency.
CHUNKS = [512, 512, 1024, 2048] + [4096] * 9
assert sum(CHUNKS) == F
MT = BC // 128  # 4 batch tiles per core

# Filled by kernel() when NNUE_TRACE=1; read by test.py.
LAST_RESULTS = None


def _build_program(ft_b_last: float, l3_b0: float):
    import concourse.bacc as bacc
    import concourse.mybir as mybir
    import concourse.tile as tile
    from concourse._compat import get_trn_type

    f16 = mybir.dt.float16
    f32 = mybir.dt.float32
    f8 = mybir.dt.float8e4
    AF = mybir.ActivationFunctionType

    nc = bacc.Bacc(
        get_trn_type() or "TRN2",
        target_bir_lowering=False,
        debug=False,
        num_devices=NCORES,
    )

    wT_d = nc.dram_tensor("wT", [F, BC], f8, kind="ExternalInput")
    bT_d = nc.dram_tensor("bT", [F, BC], f8, kind="ExternalInput")
    ftwT_d = nc.dram_tensor("ftwT", [F, O], f16, kind="ExternalInput")
    ftb_d = nc.dram_tensor("ftb", [O, 1], f32, kind="ExternalInput")
    stmh_d = nc.dram_tensor("stmh", [1, BC], f32, kind="ExternalInput")
    ident_d = nc.dram_tensor("ident", [128, 128], f16, kind="ExternalInput")
    l1wT_d = nc.dram_tensor("l1wT", [512, 32], f16, kind="ExternalInput")
    l1b_d = nc.dram_tensor("l1b", [32, 1], f32, kind="ExternalInput")
    l2wT_d = nc.dram_tensor("l2wT", [32, 32], f16, kind="ExternalInput")
    l2b_d = nc.dram_tensor("l2b", [32, 1], f32, kind="ExternalInput")
    l3wT_d = nc.dram_tensor("l3wT", [32, 1], f16, kind="ExternalInput")
    y_d = nc.dram_tensor("y", [1, BC], f32, kind="ExternalOutput")

    with tile.TileContext(nc) as tc, ExitStack() as ctx:
        const = ctx.enter_context(tc.tile_pool(name="const", bufs=1))
        wpool = ctx.enter_context(tc.tile_pool(name="wpool", bufs=3))
        bpool = ctx.enter_context(tc.tile_pool(name="bpool", bufs=3))
        fpool = ctx.enter_context(tc.tile_pool(name="fpool", bufs=3))
        epi = ctx.enter_context(tc.tile_pool(name="epi", bufs=1))
        ps = ctx.enter_context(tc.tile_pool(name="ps", bufs=8, space="PSUM"))

        # --- constants into SBUF ---
        ident = const.tile([128, 128], f16, tag="ident")
        nc.gpsimd.dma_start(ident[:], ident_d.ap())
        stmh = const.tile([1, BC], f32, tag="stmh")
        nc.gpsimd.dma_start(stmh[:], stmh_d.ap())
        ftb0 = const.tile([128, 1], f32, tag="ftb0")
        nc.gpsimd.dma_start(ftb0[:], ftb_d.ap()[0:128, :])
        ftb1 = const.tile([128, 1], f32, tag="ftb1")
        nc.gpsimd.dma_start(ftb1[:], ftb_d.ap()[128:256, :])
        l1wT = const.tile([128, 4, 32], f16, tag="l1wT")
        nc.gpsimd.dma_start(l1wT[:], l1wT_d.ap().rearrange("(s p) o -> p s o", p=128))
        l1b = const.tile([32, 1], f32, tag="l1b")
        nc.gpsimd.dma_start(l1b[:], l1b_d.ap())
        l2wT = const.tile([32, 32], f16, tag="l2wT")
        nc.gpsimd.dma_start(l2wT[:], l2wT_d.ap())
        l2b = const.tile([32, 1], f32, tag="l2b")
        nc.gpsimd.dma_start(l2b[:], l2b_d.ap())
        l3wT = const.tile([32, 1], f16, tag="l3wT")
        nc.gpsimd.dma_start(l3wT[:], l3wT_d.ap())

        # --- PE warm-up: keep TensorE busy during the DMA ramp so HAM
        # reaches K=8/8 before the first real matmul (and the ramp overlaps).
        warm = const.tile([128, 512], f16, tag="warm")
        nc.vector.memset(warm[:], 0.0)
        wps = ps.tile([128, 512], f32, tag="ps", name="warmps")
        for i in range(40):
            nc.tensor.matmul(
                wps[:], warm[:, 0:128], warm[:], start=True, stop=True
            )

        # --- feature transformer: accumulate wp/bp [512, 257] in PSUM ---
        accw = [ps.tile([128, O], f32, tag="ps", name=f"accw{m}") for m in range(MT)]
        accb = [ps.tile([128, O], f32, tag="ps", name=f"accb{m}") for m in range(MT)]

        off = 0
        nslices = F // 128
        sl_done = 0
        for ci, L in enumerate(CHUNKS):
            KS = L // 128
            ft = fpool.tile([128, KS, O], f16, tag="fchunk", name=f"ft{ci}")
            nc.sync.dma_start(
                ft[:],
                ftwT_d.ap()[off : off + L, :].rearrange("(p s) o -> p s o", s=KS),
            )
            wt = wpool.tile([128, KS, BC], f8, tag="wchunk", name=f"wt{ci}")
            nc.sync.dma_start(
                wt[:],
                wT_d.ap()[off : off + L, :].rearrange("(p s) b -> p s b", s=KS),
            )
            bt = bpool.tile([128, KS, BC], f8, tag="bchunk", name=f"bt{ci}")
            nc.sync.dma_start(
                bt[:],
                bT_d.ap()[off : off + L, :].rearrange("(p s) b -> p s b", s=KS),
            )
            for ks in range(KS):
                start = sl_done == 0
                stop = sl_done == nslices - 1
                rhs = ft[:, ks, :]
                for m in range(MT):
                    nc.tensor.matmul(
                        accw[m][:],
                        wt[:, ks, m * 128 : (m + 1) * 128],
                        rhs,
                        start=start,
                        stop=stop,
                    )
                for m in range(MT):
                    nc.tensor.matmul(
                        accb[m][:],
                        bt[:, ks, m * 128 : (m + 1) * 128],
                        rhs,
                        start=start,
                        stop=stop,
                    )
                sl_done += 1
            off += L

        # --- epilogue ---
        # Evacuate PSUM -> SBUF as fp16 (values ~ +-0.5; fp16 adds ~1e-4 rel).
        sw = [epi.tile([128, O], f16, tag=f"sw{m}", name=f"sw{m}") for m in range(MT)]
        sb = [epi.tile([128, O], f16, tag=f"sb{m}", name=f"sb{m}") for m in range(MT)]
        for m in range(MT):
            nc.scalar.copy(sw[m][:], accw[m][:])
            nc.scalar.copy(sb[m][:], accb[m][:])

        # Transpose to [out, batch] layout; fuse +ft_b and relu into the
        # PSUM->SBUF copy after each transpose.  wts/bts[h] hold relu(acc+bias)
        # for output rows h*128..h*128+127, all 512 batch columns.
        wts = [epi.tile([128, BC], f16, tag=f"wts{h}", name=f"wts{h}") for h in range(2)]
        bts = [epi.tile([128, BC], f16, tag=f"bts{h}", name=f"bts{h}") for h in range(2)]
        ftbs = [ftb0, ftb1]
        for h in range(2):
            for m in range(MT):
                tpw = ps.tile([128, 128], f16, tag="ps")
                nc.tensor.transpose(
                    tpw[:], sw[m][:, h * 128 : (h + 1) * 128], ident[:]
                )
                nc.scalar.activation(
                    wts[h][:, m * 128 : (m + 1) * 128],
                    tpw[:],
                    AF.Relu,
                    bias=ftbs[h][:],
                )
                tpb = ps.tile([128, 128], f16, tag="ps")
                nc.tensor.transpose(
                    tpb[:], sb[m][:, h * 128 : (h + 1) * 128], ident[:]
                )
                nc.scalar.activation(
                    bts[h][:, m * 128 : (m + 1) * 128],
                    tpb[:],
                    AF.Relu,
                    bias=ftbs[h][:],
                )

        # PSQT column (out idx 256) -> [1, 512] rows (keep f32).
        wqs = epi.tile([1, BC], f32, tag="wqs")
        bqs = epi.tile([1, BC], f32, tag="bqs")
        for m in range(MT):
            tq = ps.tile([1, 128], f16, tag="ps")
            nc.tensor.transpose(tq[:], sw[m][:, 256:257], ident[:])
            nc.scalar.copy(wqs[:, m * 128 : (m + 1) * 128], tq[:])
            tq2 = ps.tile([1, 128], f16, tag="ps")
            nc.tensor.transpose(tq2[:], sb[m][:, 256:257], ident[:])
            nc.scalar.copy(bqs[:, m * 128 : (m + 1) * 128], tq2[:])

        # Host already applied the stm swap (wT holds the stm-side mask,
        # bT the other side), so x0 = [wts | bts] directly; just clip to 1.
        x0 = [wts[0], wts[1], bts[0], bts[1]]
        for k in range(4):
            nc.vector.tensor_scalar_min(x0[k][:], x0[k][:], 1.0)

        # l1: [32, 512] = l1_w [32,512] @ x0 [512, 512b]  (fp16 operands)
        p1 = ps.tile([32, BC], f32, tag="ps")
        for k in range(4):
            nc.tensor.matmul(
                p1[:], l1wT[:, k, :], x0[k][:], start=(k == 0), stop=(k == 3)
            )
        x1 = epi.tile([32, BC], f16, tag="x1")
        nc.scalar.activation(x1[:], p1[:], AF.Relu, bias=l1b[:])
        nc.vector.tensor_scalar_min(x1[:], x1[:], 1.0)

        # l2: [32, 512]
        p2 = ps.tile([32, BC], f32, tag="ps")
        nc.tensor.matmul(p2[:], l2wT[:], x1[:], start=True, stop=True)
        x2 = epi.tile([32, BC], f16, tag="x2")
        nc.scalar.activation(x2[:], p2[:], AF.Relu, bias=l2b[:])
        nc.vector.tensor_scalar_min(x2[:], x2[:], 1.0)

        # l3: [1, 512] + l3_b
        p3 = ps.tile([1, BC], f32, tag="ps")
        nc.tensor.matmul(p3[:], l3wT[:], x2[:], start=True, stop=True)
        x3 = epi.tile([1, BC], f32, tag="x3")
        nc.scalar.copy(x3[:], p3[:])
        nc.vector.tensor_scalar_add(x3[:], x3[:], l3_b0)

        # + (wpsqt + bpsqt + 2*ft_b[256]) * (stm - 0.5)
        q = epi.tile([1, BC], f32, tag="q")
        nc.vector.tensor_add(q[:], wqs[:], bqs[:])
        nc.vector.tensor_scalar_add(q[:], q[:], 2.0 * ft_b_last)
        nc.vector.tensor_mul(q[:], q[:], stmh[:])
        yout = epi.tile([1, BC], f32, tag="yout")
        nc.vector.tensor_add(yout[:], x3[:], q[:])
        nc.sync.dma_start(y_d.ap(), yout[:])

    nc.compile()
    return nc


def kernel(wfts, bfts, stm, ft_w, ft_b, l1_w, l1_b, l2_w, l2_b, l3_w, l3_b):
    global LAST_RESULTS
    from concourse import bass_utils

    trace = os.environ.get("NNUE_TRACE") == "1"
    if trace:
        bass_utils.upload_artifacts = lambda tmpdir: tmpdir

    nc = _build_program(float(ft_b[O - 1]), float(l3_b[0]))

    # --- host-side shard + layout prep ---
    # Per feature-chunk [off, off+L): row p*KS+s of the chunk block holds
    # feature off+s*128+p, so each SBUF partition's DMA source is one
    # contiguous KS*ncol run (single large descriptor per partition).
    def chunk_permute(a_t):
        # a_t: [F, ncol] (feature-major); returns same shape, rows permuted
        ncol = a_t.shape[1]
        out = np.empty_like(a_t)
        off = 0
        for L in CHUNKS:
            ks = L // 128
            blk = a_t[off : off + L].reshape(ks, 128, ncol)
            out[off : off + L] = np.ascontiguousarray(
                blk.transpose(1, 0, 2)
            ).reshape(L, ncol)
            off += L
        return out

    ftwT = chunk_permute(np.ascontiguousarray(ft_w.T.astype(np.float16)))  # [F, 257]
    ftb = np.ascontiguousarray(ft_b.reshape(O, 1)).astype(np.float32)
    ident = np.eye(128, dtype=np.float16)
    l1wT = np.ascontiguousarray(l1_w.T).astype(np.float16)  # [512, 32]
    l1bc = np.ascontiguousarray(l1_b.reshape(32, 1)).astype(np.float32)
    l2wT = np.ascontiguousarray(l2_w.T).astype(np.float16)
    l2bc = np.ascontiguousarray(l2_b.reshape(32, 1)).astype(np.float32)
    l3wT = np.ascontiguousarray(l3_w.T).astype(np.float16)  # [32, 1]

    import ml_dtypes

    wfts16 = wfts.astype(ml_dtypes.float8_e4m3)  # 0/1 exact in fp8
    bfts16 = bfts.astype(ml_dtypes.float8_e4m3)

    in_maps = []
    for c in range(NCORES):
        sl = slice(c * BC, (c + 1) * BC)
        stm_c = stm[sl, 0].astype(np.float32)
        pick = stm_c[:, None] > 0.5
        m1 = np.where(pick, wfts16[sl, :], bfts16[sl, :])  # stm side
        m2 = np.where(pick, bfts16[sl, :], wfts16[sl, :])  # other side
        wT = chunk_permute(np.ascontiguousarray(m1.T))  # [F, 512]
        bT = chunk_permute(np.ascontiguousarray(m2.T))
        stmh = np.ascontiguousarray((stm_c - 0.5)[None, :])
        in_maps.append(
            {
                "wT": wT,
                "bT": bT,
                "ftwT": ftwT,
                "ftb": ftb,
                "stmh": stmh,
                "ident": ident,
                "l1wT": l1wT,
                "l1b": l1bc,
                "l2wT": l2wT,
                "l2b": l2bc,
                "l3wT": l3wT,
            }
        )

    res = bass_utils.run_bass_kernel_spmd(
        nc, in_maps, core_ids=list(range(NCORES)), trace=trace
    )
    if trace:
        LAST_RESULTS = res

    out = np.empty((B, 1), dtype=np.float32)
    for c in range(NCORES):
        out[c * BC : (c + 1) * BC, 0] = res.results[c]["y"][0]
    return out

